# revision 34
# baseline (speedup 1.0000x reference)
"""Trainium2 Bass kernel for dual channel-attention block (nn_Attention_85985245266248).

Strategy (the wall-clock bottleneck is the ~40-55MB/s axon tunnel, so the
design minimizes bytes-per-dispatch and sharded-array count):
  - Shard spatially: 256 rows -> 8 cores x 32 rows (exact, no halo shipped).
  - ONE flat int8 blob input per core (~3.3MB): int8 x-shards with
    per-(batch,channel) scales + 1/8th of the bf16 weights + bitcast f32
    smalls. Each sharded array costs ~70ms of fixed axon latency, so
    everything is packed into a single tensor.
  - ONE int8 blob output per core: payload quantized on device to int8 with
    per-(core,batch,channel) scales (round-to-nearest-even), the f32 scales
    bitcast into 4 extra bytes per channel row. Halves both the donated
    zero-buffer upload and the output download.
  - Weights AllGather'd on device from the 1/8th shards.
  - Halo rows exchanged on device: AllGather of boundary rows, then each
    core picks neighbors' rows via runtime-offset DMA (partition_id) from a
    zero-guarded DRAM buffer (global edges read zeros).
  - conv1x1 + depthwise3x3 folded into a full 3x3 conv (rank-1 weights),
    executed as 9 PSUM-accumulated matmuls per tile on the PE.
  - Pass A computes q,k in [px, ch] layout (input stationary, weights moving)
    so the c-x-c Gram matrices q@k^T and the L2 norms come straight off the
    PE with pixel-contraction; partial Grams are AllReduce'd across cores.
  - Pass B computes v in [ch, px] layout (weights stationary).
  - Softmax + norm scaling on DVE/ACT (tiny 96x96 tensors).
  - Output projection po/concat folded on host into P_c/P_t; final output is
    two accumulated matmuls per pixel chunk: out = M_cT^T @ v_t + M_tT^T @ v_c + b.
All heavy matmuls run in bf16 (fp32 accumulate in PSUM). End-to-end rel err
~1.2e-2 vs the f32 reference (gate 2e-2), dominated by input int8 quant.
"""
import os
import sys
import numpy as np

sys.path.insert(0, "/opt/trn_rl_repo")

# Persistent XLA compilation cache: run_bass_kernel_spmd builds a fresh jit
# closure per call, which defeats jax's in-memory executable cache and would
# re-run the NEFF compile hook on every dispatch (~0.5s). The disk cache
# makes repeat dispatches hit.
import jax as _jax
_jax.config.update("jax_compilation_cache_dir", "/tmp/jax_comp_cache")
_jax.config.update("jax_persistent_cache_min_compile_time_secs", 0.0)
_jax.config.update("jax_persistent_cache_min_entry_size_bytes", 0)

B = 2
D = 96
H = 256
W = 256
HEADS = 3
NC = 8
RPC = H // NC          # rows per core = 32
HR = RPC + 2           # halo rows = 34
PW = W + 2             # padded width = 258
PXT = 128              # pass-A pixel tile (half row)
NT_A = RPC * W // PXT  # pass-A tiles per batch per tensor = 64
CHK = 512              # pass-B / final chunk = 2 rows
NCHK = RPC * W // CHK  # 16

# flat bf16 weight-gather layout: (name, elems)
WPACK = [
    ("wqk_hi", D * 9 * 2 * D),   # 165888
    ("wqk_lo", D * 9 * 2 * D),   # 165888
    ("wv_hi", D * 9 * D),        # 82944
    ("wv_lo", D * 9 * D),        # 82944
    ("pct", D * D),              # 9216
    ("ptt", D * D),              # 9216
    ("ident", D * D),            # 9216
]
WTOT = sum(n for _, n in WPACK)  # 525312
WSH = WTOT // NC                 # 65664 per core

# single-blob input layout (bytes). One sharded array per dispatch kills
# the ~70ms-per-array axon fixed cost. Halo rows are exchanged on-device
# (AllGather of boundary rows + dynamic-offset DMA), not shipped.
# x ships as int7 (8 values packed into 7 bytes, per-row scales): group g
# bytes b_1..b_7 carry v_1..v_7 in their low 7 bits and bit (i-1) of v_0
# in b_i's MSB. Unpacked on DVE with pure arithmetic (is_lt/is_ge/mult/add).
WPB = W // 8 * 7               # packed bytes per row = 224
XP1 = D * RPC * WPB            # one (tensor, batch) packed block = 688128
WOFF = 4 * XP1                 # bf16 weight shard bytes
SOFF = WOFF + WSH * 2          # f32 row scales [4, D, RPC] then tempv, biasv
SCB = D * RPC * 4              # row-scale bytes per block = 12288
TOFF = SOFF + 4 * SCB
NB = TOFF + 2 * D * 4          # total blob bytes per core
OCOL = RPC * W + 4             # int8 payload + bitcast f32 scale per row
# halo contribution: 4 blocks x {top,bot} packed rows + 4 x 2 f32 row scales
CBS = 4 * 2 * WPB              # = 1792 (scale bytes follow)
CBC = CBS + 4 * 2 * 4          # = 1824

_CACHE = {}


def _fold3x3(w1, dw):
    """w1:[O,C], dw:[O,1,3,3] -> [9, C, O] rhs-layout folded weights."""
    O, C = w1.shape
    out = np.zeros((9, C, O), np.float32)
    for t in range(9):
        dy, dx = t // 3, t % 3
        out[t] = (dw[:, 0, dy, dx][:, None] * w1).T
    return out


def _bf16(a):
    import ml_dtypes
    return np.asarray(a, np.float32).astype(ml_dtypes.bfloat16)


def _build(nc_mod):
    """Build the Bass program (uses modules passed in)."""
    bass, bacc, tile, mybir = nc_mod
    from concourse import bass_isa
    f32 = mybir.dt.float32
    bf16 = mybir.dt.bfloat16
    i8 = mybir.dt.int8

    nc = bacc.Bacc("TRN2", target_bir_lowering=False, debug=False, num_devices=NC)

    # I/O: one flat int8 blob in (x shards + bf16 weight shard + f32 smalls,
    # all bitcast), one int8 blob out (payload + bitcast f32 scale per row).
    xin = nc.dram_tensor("xin", [1, NB], i8, kind="ExternalInput")
    oq8 = nc.dram_tensor("oq8", [B, D, OCOL], i8, kind="ExternalOutput")

    NG = 6  # grams per batch: G1, G2, Sqc, Skc, Sqt, Skt

    with tile.TileContext(nc) as tc:
        with (
            tc.tile_pool(name="consts", bufs=1) as cpool,
            tc.tile_pool(name="xq", bufs=1) as xqpool,
            tc.tile_pool(name="xres", bufs=1) as xpool,
            tc.tile_pool(name="vres", bufs=1) as vpool,
            tc.tile_pool(name="qk", bufs=4) as qkpool,
            tc.tile_pool(name="work_ps", bufs=3, space="PSUM") as wps,
            tc.tile_pool(name="gram_ps", bufs=1, space="PSUM") as gps,
            tc.tile_pool(name="small", bufs=1) as spool,
            tc.tile_pool(name="obuf", bufs=1) as opool,
            tc.tile_pool(name="dram", bufs=1, space="DRAM") as dpool,
        ):
            # ---- weight AllGather: 1/8th slice per core -> full flat ----
            wsh_sb = cpool.tile([D, WSH // D], bf16, tag="wsh")
            nc.sync.dma_start(out=wsh_sb[:],
                              in_=xin[0, WOFF:WOFF + WSH * 2].bitcast(bf16))
            wag_in = dpool.tile([1, WSH], bf16, tag="wagin")
            wag_out = dpool.tile([1, WTOT], bf16, tag="wagout")
            nc.gpsimd.dma_start(out=wag_in[:], in_=wsh_sb[:])
            nc.gpsimd.collective_compute(
                "AllGather",
                mybir.AluOpType.bypass,
                replica_groups=[list(range(NC))],
                ins=[wag_in.opt()],
                outs=[wag_out.opt()],
            )

            # ---- unpack gathered weights into const tiles ----
            wqk_hi_sb = cpool.tile([D, 9, 2 * D], bf16, tag="wqkh")
            wqk_lo_sb = cpool.tile([D, 9, 2 * D], bf16, tag="wqkl")
            wv_hi_sb = cpool.tile([D, 9, D], bf16, tag="wvh")
            wv_lo_sb = cpool.tile([D, 9, D], bf16, tag="wvl")
            pct_sb = cpool.tile([D, D], bf16, tag="pct")
            ptt_sb = cpool.tile([D, D], bf16, tag="ptt")
            identb_sb = cpool.tile([D, D], bf16, tag="identb")
            wtiles = {"wqk_hi": wqk_hi_sb, "wqk_lo": wqk_lo_sb,
                      "wv_hi": wv_hi_sb, "wv_lo": wv_lo_sb,
                      "pct": pct_sb, "ptt": ptt_sb, "ident": identb_sb}
            off = 0
            for nm, n in WPACK:
                nc.gpsimd.dma_start(out=wtiles[nm][:], in_=wag_out[0, off:off + n])
                off += n
            ident_sb = cpool.tile([D, D], f32, tag="ident")
            nc.vector.tensor_copy(ident_sb[:], identb_sb[:])

            scrow_sb = []
            for idx in range(4):
                t = cpool.tile([D, RPC], f32, tag=f"scr{idx}")
                o = SOFF + idx * SCB
                nc.sync.dma_start(out=t[:], in_=xin[0, o:o + SCB].bitcast(f32))
                scrow_sb.append(t)
            tempv_sb = cpool.tile([D, 1], f32, tag="tempv")
            biasv_sb = cpool.tile([D, 1], f32, tag="biasv")
            nc.sync.dma_start(out=tempv_sb[:],
                              in_=xin[0, TOFF:TOFF + D * 4].bitcast(f32))
            nc.sync.dma_start(out=biasv_sb[:],
                              in_=xin[0, TOFF + D * 4:NB].bitcast(f32))

            # ---- halo exchange: AllGather packed boundary rows + row scales,
            # then each core fetches its neighbors' full contributions via
            # runtime-offset DMA from a zero-guard-padded buffer.
            cb = spool.tile([D, CBC], i8, tag="cb")
            for si in range(2):
                for b in range(B):
                    idx = si * B + b
                    blk = xin[0, idx * XP1:(idx + 1) * XP1].rearrange(
                        "(c r w) -> c r w", c=D, r=RPC, w=WPB)
                    c0 = idx * 2 * WPB
                    nc.sync.dma_start(out=cb[:, c0:c0 + WPB], in_=blk[:, 0, :])
                    nc.sync.dma_start(out=cb[:, c0 + WPB:c0 + 2 * WPB],
                                      in_=blk[:, RPC - 1, :])
                    so = SOFF + idx * SCB
                    sc = xin[0, so:so + SCB].rearrange(
                        "(c r q) -> c r q", c=D, r=RPC, q=4)
                    d0 = CBS + idx * 8
                    nc.sync.dma_start(out=cb[:, d0:d0 + 4], in_=sc[:, 0, :])
                    nc.sync.dma_start(out=cb[:, d0 + 4:d0 + 8],
                                      in_=sc[:, RPC - 1, :])
            hg_in = dpool.tile([D, CBC], i8, tag="hgin")
            hg_out = dpool.tile([NC * D, CBC], i8, tag="hgout")
            nc.gpsimd.dma_start(out=hg_in[:], in_=cb[:])
            nc.gpsimd.collective_compute(
                "AllGather",
                mybir.AluOpType.bypass,
                replica_groups=[list(range(NC))],
                ins=[hg_in.opt()],
                outs=[hg_out.opt()],
            )
            # pad with zeroed guard blocks so pid 0/7 reads hit zeros (the
            # reference zero-pads at global edges) and offsets stay in range
            hg_pad = dpool.tile([(NC + 2) * D, CBC], i8, tag="hgpad")
            zrow = spool.tile([D, CBC], i8, tag="zrow")
            nc.vector.memset(zrow[:], 0.0)
            nc.sync.dma_start(out=hg_pad[0:D], in_=zrow[:])
            nc.sync.dma_start(out=hg_pad[(NC + 1) * D:(NC + 2) * D], in_=zrow[:])
            nc.gpsimd.dma_start(out=hg_pad[D:(NC + 1) * D], in_=hg_out[:])
            pid = nc.sync.partition_id()
            nb_up = spool.tile([D, CBC], i8, tag="nbup")
            nb_dn = spool.tile([D, CBC], i8, tag="nbdn")
            nc.sync.dma_start(out=nb_up[:], in_=hg_pad[bass.ds(pid * D, D), :])
            nc.sync.dma_start(out=nb_dn[:],
                              in_=hg_pad[bass.ds((pid + 2) * D, D), :])

            # gram accumulation targets and per-batch v stores
            gram_cat = spool.tile([D, B * NG * D], f32, tag="gramcat")
            v_sb = {}   # (b, 'hi'/'lo') -> [D, RPC*W] bf16
            for b in range(B):
                for s in ("hi", "lo"):
                    v_sb[(b, s)] = vpool.tile([D, RPC * W], bf16,
                                              tag=f"v{b}{s}", name=f"v{b}{s}")

            xt = {}
            for b in range(B):
                # ---- load, unpack int7->int8, dequantize per row ----
                AL = mybir.AluOpType

                def unpack7(get_src, get_dst, sshape, tagp):
                    """int7 unpack: 7 sign-extends + MSB-scatter reassembly of
                    v0, using only is_lt/is_ge/mult/add (DVE fp32 internally).
                    get_src(i)/get_dst(i): stream-i APs; sshape: scratch shape."""
                    acc = spool.tile(sshape, f32, tag=f"u7a{tagp}")
                    neg = spool.tile(sshape, f32, tag=f"u7n{tagp}")
                    low = spool.tile(sshape, f32, tag=f"u7l{tagp}")
                    ge = spool.tile(sshape, f32, tag=f"u7g{tagp}")
                    for i in range(1, 8):
                        src = get_src(i)
                        nc.vector.tensor_scalar(out=neg[:], in0=src, scalar1=0,
                                                scalar2=None, op0=AL.is_lt)
                        nc.vector.tensor_scalar(out=low[:], in0=neg[:],
                                                scalar1=128.0, scalar2=None,
                                                op0=AL.mult)
                        nc.vector.tensor_tensor(out=low[:], in0=low[:], in1=src,
                                                op=AL.add)
                        nc.vector.tensor_scalar(out=ge[:], in0=low[:], scalar1=64,
                                                scalar2=None, op0=AL.is_ge)
                        nc.vector.tensor_scalar(out=ge[:], in0=ge[:],
                                                scalar1=-128.0, scalar2=None,
                                                op0=AL.mult)
                        nc.vector.tensor_tensor(out=get_dst(i), in0=low[:],
                                                in1=ge[:], op=AL.add)
                        if i == 1:
                            nc.vector.tensor_copy(acc[:], neg[:])
                        else:
                            nc.vector.tensor_scalar(out=neg[:], in0=neg[:],
                                                    scalar1=float(1 << (i - 1)),
                                                    scalar2=None, op0=AL.mult)
                            nc.vector.tensor_tensor(out=acc[:], in0=acc[:],
                                                    in1=neg[:], op=AL.add)
                    nc.vector.tensor_scalar(out=ge[:], in0=acc[:], scalar1=64,
                                            scalar2=None, op0=AL.is_ge)
                    nc.vector.tensor_scalar(out=ge[:], in0=ge[:], scalar1=-128.0,
                                            scalar2=None, op0=AL.mult)
                    nc.vector.tensor_tensor(out=get_dst(0), in0=acc[:],
                                            in1=ge[:], op=AL.add)

                for si, s in enumerate(("hi", "lo")):
                    idx = si * B + b
                    xp = xqpool.tile([D, RPC, WPB], i8, tag="xp")
                    nc.sync.dma_start(out=xp[:],
                                      in_=xin[0, idx * XP1:(idx + 1) * XP1])
                    xq = xqpool.tile([D, HR, PW], i8, tag="xq")
                    nc.vector.memset(xq[:], 0.0)
                    # interior rows 1..32
                    unpack7(
                        lambda i: xp[:, :, i - 1::7] if i else None,
                        lambda i: xq[:, 1:RPC + 1, 1 + i:W + 1:8],
                        [D, RPC, W // 8], "m")
                    # halo rows: neighbor's packed boundary rows
                    c0 = idx * 2 * WPB
                    unpack7(
                        lambda i: nb_up[:, c0 + WPB + i - 1:c0 + 2 * WPB:7],
                        lambda i: xq[:, 0, 1 + i:W + 1:8],
                        [D, W // 8], "h")
                    unpack7(
                        lambda i: nb_dn[:, c0 + i - 1:c0 + WPB:7],
                        lambda i: xq[:, HR - 1, 1 + i:W + 1:8],
                        [D, W // 8], "h")
                    # per-row dequant (halo rows use the neighbor's row scale)
                    xd = xpool.tile([D, HR, PW], bf16, tag=f"x{s}")
                    nc.vector.memset(xd[:], 0.0)
                    d0 = CBS + idx * 8
                    for r in range(HR):
                        if r == 0:
                            sap = nb_up[:, d0 + 4:d0 + 8].bitcast(f32)
                        elif r == HR - 1:
                            sap = nb_dn[:, d0:d0 + 4].bitcast(f32)
                        else:
                            sap = scrow_sb[idx][:, r - 1:r]
                        nc.scalar.activation(
                            xd[:, r, 1:W + 1], xq[:, r, 1:W + 1],
                            mybir.ActivationFunctionType.Identity,
                            bias=0.0, scale=sap)
                    xt[(b, s)] = xd
                    del xp, xq, xd

                # ---- pass A: q,k in [px, ch] + Gram/norm accumulation ----
                # paired layout sbp[:, g, :]: g=0 -> [q_c | k_t], g=1 -> [k_c | q_t]
                gA = gps.tile([D, 2 * D], f32, tag="gA", name=f"gA{b}")  # [Sqc | G1]
                gB = gps.tile([D, 2 * D], f32, tag="gB", name=f"gB{b}")  # [G2 | Sqt]
                gC = gps.tile([D, D], f32, tag="gC", name=f"gC{b}")      # Skt
                gD = gps.tile([D, D], f32, tag="gD", name=f"gD{b}")      # Skc

                def grams(sbp, first, last):
                    nc.tensor.matmul(gA[:], sbp[:, 0, 0:D], sbp[:, 0, :],
                                     start=first, stop=last)
                    nc.tensor.matmul(gB[:], sbp[:, 1, D:2 * D], sbp[:, 1, :],
                                     start=first, stop=last)
                    nc.tensor.matmul(gC[:], sbp[:, 0, D:2 * D], sbp[:, 0, D:2 * D],
                                     start=first, stop=last)
                    nc.tensor.matmul(gD[:], sbp[:, 1, 0:D], sbp[:, 1, 0:D],
                                     start=first, stop=last)

                prev = None
                for it in range(NT_A):
                    r = (it * PXT) // W          # output row 0..31
                    j = (it * PXT) % W           # 0 or 128
                    sbp = qkpool.tile([PXT, 2, 2 * D], bf16, tag="qksb")
                    for gi, (s, wsb) in enumerate((("hi", wqk_hi_sb),
                                                   ("lo", wqk_lo_sb))):
                        ps = wps.tile([PXT, 2 * D], f32, tag="apsum")
                        xs = xt[(b, s)]
                        for t in range(9):
                            dy, dx = t // 3, t % 3
                            lhsT = xs[:, r + dy, j + dx:j + dx + PXT]
                            nc.tensor.matmul(ps[:], lhsT, wsb[:, t, :],
                                             start=(t == 0), stop=(t == 8))
                        # hi [q_c|k_c] -> cols {0:96, 192:288}; lo [k_t|q_t] -> {96:192, 288:384}
                        nc.vector.tensor_copy(sbp[:, :, gi * D:(gi + 1) * D], ps[:])
                    if prev is not None:
                        grams(prev, prev_first, False)
                    prev_first = prev is None
                    prev = sbp
                grams(prev, False, True)

                for k, src in (("G1", gA[:, D:2 * D]), ("G2", gB[:, 0:D]),
                               ("Sqc", gA[:, 0:D]), ("Skc", gD[:]),
                               ("Sqt", gB[:, D:2 * D]), ("Skt", gC[:])):
                    gi = ("G1", "G2", "Sqc", "Skc", "Sqt", "Skt").index(k)
                    off = (b * NG + gi) * D
                    nc.vector.tensor_copy(gram_cat[:, off:off + D], src)

                # ---- pass B: v in [ch, px] ----
                for s, wsb in (("hi", wv_hi_sb), ("lo", wv_lo_sb)):
                    xs = xt[(b, s)]
                    for ck in range(NCHK):
                        r = ck * 2
                        ps = wps.tile([D, CHK], f32, tag="apsum")
                        for t in range(9):
                            dy, dx = t // 3, t % 3
                            rhs = xs[:, r + dy:r + dy + 2, dx:dx + W]
                            nc.tensor.matmul(ps[:], wsb[:, t, :], rhs,
                                             start=(t == 0), stop=(t == 8))
                        nc.vector.tensor_copy(
                            v_sb[(b, s)][:, ck * CHK:(ck + 1) * CHK], ps[:])

            # ---- AllReduce partial grams across the 8 cores ----
            ar_in = dpool.tile([D, B * NG * D], f32, tag="arin")
            ar_out = dpool.tile([D, B * NG * D], f32, tag="arout")
            nc.gpsimd.dma_start(out=ar_in[:], in_=gram_cat[:])
            nc.gpsimd.collective_compute(
                "AllReduce",
                mybir.AluOpType.add,
                replica_groups=[list(range(NC))],
                ins=[ar_in.opt()],
                outs=[ar_out.opt()],
            )
            gram_red = spool.tile([D, B * NG * D], f32, tag="gramred")
            nc.gpsimd.dma_start(out=gram_red[:], in_=ar_out[:])

            # ---- post-AR small compute per batch ----
            mt = {}  # (b, 'c'/'t') -> M^T tile [D, D] bf16
            for b in range(B):
                def gslice(gi):
                    off = (b * NG + gi) * D
                    return gram_red[:, off:off + D]
                G1, G2, Sqc, Skc, Sqt, Skt = [gslice(i) for i in range(NG)]

                rcol = {}
                for nm, S in (("qc", Sqc), ("kc", Skc), ("qt", Sqt), ("kt", Skt)):
                    tmp = spool.tile([D, D], f32, tag="dtmp")
                    nc.vector.tensor_tensor(out=tmp[:], in0=S, in1=ident_sb[:],
                                            op=mybir.AluOpType.mult)
                    dg = spool.tile([D, 1], f32, tag=f"d{nm}{b}")
                    nc.vector.tensor_reduce(out=dg[:], in_=tmp[:],
                                            axis=mybir.AxisListType.X,
                                            op=mybir.AluOpType.add)
                    sq = spool.tile([D, 1], f32, tag=f"sq{nm}{b}")
                    nc.scalar.sqrt(sq[:], dg[:])
                    rc = spool.tile([D, 1], f32, tag=f"rc{nm}{b}")
                    nc.vector.reciprocal(rc[:], sq[:])
                    rcol[nm] = rc
                # fold temperature into rq
                for nm in ("qc", "qt"):
                    nc.vector.tensor_tensor(out=rcol[nm][:], in0=rcol[nm][:],
                                            in1=tempv_sb[:],
                                            op=mybir.AluOpType.mult)

                # row-broadcast 1/||k||: partition all-reduce of (S*I) then rsqrt
                rrow = {}
                for nm, S in (("kt", Skt), ("kc", Skc)):
                    tmp = spool.tile([D, D], f32, tag="dtmp")
                    nc.vector.tensor_tensor(out=tmp[:], in0=S, in1=ident_sb[:],
                                            op=mybir.AluOpType.mult)
                    dall = spool.tile([D, D], f32, tag=f"da{nm}{b}")
                    nc.gpsimd.partition_all_reduce(dall[:], tmp[:], channels=D,
                                                   reduce_op=bass_isa.ReduceOp.add)
                    sq = spool.tile([D, D], f32, tag=f"sq2{nm}{b}")
                    nc.scalar.sqrt(sq[:], dall[:])
                    rb = spool.tile([D, D], f32, tag=f"rb{nm}{b}")
                    nc.vector.reciprocal(rb[:], sq[:])
                    rrow[nm] = rb

                for attn_nm, G, rq, rkb, psb in (
                        ("c", G1, rcol["qc"], rrow["kt"], pct_sb),
                        ("t", G2, rcol["qt"], rrow["kc"], ptt_sb)):
                    L = spool.tile([D, D], f32, tag=f"L{attn_nm}{b}")
                    nc.vector.tensor_scalar(out=L[:], in0=G, scalar1=rq[:],
                                            scalar2=None,
                                            op0=mybir.AluOpType.mult)
                    nc.vector.tensor_tensor(out=L[:], in0=L[:], in1=rkb[:],
                                            op=mybir.AluOpType.mult)
                    A = spool.tile([D, D], bf16, tag=f"A{attn_nm}{b}")
                    nc.vector.memset(A[:], 0.0)
                    for h in range(HEADS):
                        p0 = 32 * h
                        blk = L[p0:p0 + 32, p0:p0 + 32]
                        nmax = spool.tile([32, 1], f32, tag=f"nm{attn_nm}{b}{h}")
                        nc.vector.tensor_reduce(out=nmax[:], in_=blk,
                                                axis=mybir.AxisListType.X,
                                                op=mybir.AluOpType.max,
                                                negate=True)
                        e = spool.tile([32, 32], f32, tag=f"e{attn_nm}{b}{h}")
                        nc.scalar.activation(e[:], blk,
                                             mybir.ActivationFunctionType.Exp,
                                             bias=nmax[:], scale=1.0)
                        ssum = spool.tile([32, 1], f32, tag=f"ss{attn_nm}{b}{h}")
                        nc.vector.tensor_reduce(out=ssum[:], in_=e[:],
                                                axis=mybir.AxisListType.X,
                                                op=mybir.AluOpType.add)
                        rs = spool.tile([32, 1], f32, tag=f"rs{attn_nm}{b}{h}")
                        nc.vector.reciprocal(rs[:], ssum[:])
                        nc.vector.tensor_scalar(out=A[p0:p0 + 32, p0:p0 + 32],
                                                in0=e[:], scalar1=rs[:],
                                                scalar2=None,
                                                op0=mybir.AluOpType.mult)
                    # M^T = A(lhsT) . P^T  -> [d, o]
                    mps = wps.tile([D, D], f32, tag="apsum")
                    nc.tensor.matmul(mps[:], A[:], psb[:], start=True, stop=True)
                    msb = spool.tile([D, D], bf16, tag=f"m{attn_nm}{b}")
                    nc.vector.tensor_copy(msb[:], mps[:])
                    mt[(b, attn_nm)] = msb

            # ---- final: out = M_cT^T @ v_t + M_tT^T @ v_c + bias ----
            # Stage per-batch output in SBUF (bf16), track per-channel
            # absmax, then quantize to int8 with per-channel scale.
            for b in range(B):
                ob = opool.tile([D, RPC * W], bf16, tag="ob", name=f"ob{b}")
                amax = spool.tile([D, 1], f32, tag=f"amax{b}")
                for ck in range(NCHK):
                    ps = wps.tile([D, CHK], f32, tag="apsum")
                    sl = slice(ck * CHK, (ck + 1) * CHK)
                    nc.tensor.matmul(ps[:], mt[(b, "c")][:], v_sb[(b, "lo")][:, sl],
                                     start=True, stop=False)
                    nc.tensor.matmul(ps[:], mt[(b, "t")][:], v_sb[(b, "hi")][:, sl],
                                     start=False, stop=True)
                    nc.scalar.activation(ob[:, sl], ps[:],
                                         mybir.ActivationFunctionType.Identity,
                                         bias=biasv_sb[:], scale=1.0)
                    oabs = spool.tile([D, CHK], f32, tag="oabs")
                    nc.scalar.activation(oabs[:], ps[:],
                                         mybir.ActivationFunctionType.Abs,
                                         bias=biasv_sb[:], scale=1.0)
                    cmax = spool.tile([D, 1], f32, tag=f"cmax{b}")
                    nc.vector.tensor_reduce(out=cmax[:], in_=oabs[:],
                                            axis=mybir.AxisListType.X,
                                            op=mybir.AluOpType.max)
                    if ck == 0:
                        nc.vector.tensor_copy(amax[:], cmax[:])
                    else:
                        nc.vector.tensor_tensor(out=amax[:], in0=amax[:],
                                                in1=cmax[:],
                                                op=mybir.AluOpType.max)
                # scale = amax/127 (host dequant), rscale = 127/amax
                scl = spool.tile([D, 1], f32, tag=f"scl{b}")
                nc.vector.tensor_scalar(out=scl[:], in0=amax[:],
                                        scalar1=1.0 / 127.0, scalar2=None,
                                        op0=mybir.AluOpType.mult)
                nc.sync.dma_start(out=oq8[b][:, RPC * W:OCOL],
                                  in_=scl[:].bitcast(i8))
                rsc = spool.tile([D, 1], f32, tag=f"rsc{b}")
                nc.vector.reciprocal(rsc[:], scl[:])
                oq = opool.tile([D, RPC * W], i8, tag="oq", name=f"oq{b}")
                nc.scalar.activation(oq[:], ob[:],
                                     mybir.ActivationFunctionType.Identity,
                                     bias=0.0, scale=rsc[:])
                nc.sync.dma_start(out=oq8[b][:, 0:RPC * W], in_=oq[:])

    nc.compile()
    return nc


def _get_nc():
    if "nc" not in _CACHE:
        from concourse import bass, bacc, tile, mybir
        _CACHE["mods"] = (bass, bacc, tile, mybir)
        nc = _build(_CACHE["mods"])
        # memoize the (pure, deterministic) BIR serialization: the bass2jax
        # lowering re-serializes it on every dispatch (~40ms for 4.7MB)
        jb = nc.to_json_bytes()
        nc.to_json_bytes = lambda: jb
        _CACHE["nc"] = nc
    return _CACHE["nc"]


def _pack7(v):
    """v: [..., 8k] int8 in [-63,63] -> [..., 7k] MSB-scatter packed."""
    sh = v.shape[:-1]
    g = v.reshape(*sh, -1, 8).astype(np.uint8)
    b = np.zeros((*sh, g.shape[-2], 7), np.uint8)
    v0 = g[..., 0] & 0x7F
    for i in range(1, 8):
        b[..., i - 1] = (g[..., i] & 0x7F) | (((v0 >> (i - 1)) & 1) << 7)
    return b.reshape(*sh, -1).view(np.int8)


def _quant_in(x):
    """x: [B,D,H,W] f32 -> (packed int7 shards per core, row scales [B,D,H])."""
    x = np.asarray(x, np.float32)
    sc = np.maximum(np.abs(x).max(axis=3, keepdims=True) / 63.0, 1e-30)
    xq = np.clip(np.round(x / sc), -63, 63).astype(np.int8)
    xp = _pack7(xq)  # [B,D,H,WPB]
    sh = [np.ascontiguousarray(xp[:, :, c * RPC:(c + 1) * RPC, :])
          for c in range(NC)]
    return sh, np.ascontiguousarray(sc[:, :, :, 0])


def _prep_inputs(low, high, temperature, qc_w, qdw_c_w, kvc_w, kvdw_c_w,
                 qt_w, qdw_t_w, kvt_w, kvdw_t_w, po_c_w, po_t_w,
                 concat_w, concat_b):
    """Host-side weight folding + input shard/pad/quant. Returns in_maps."""
    W3 = {
        "q_hi": _fold3x3(qc_w, qdw_c_w),
        "k_hi": _fold3x3(kvc_w[:96], kvdw_c_w[:96]),
        "v_hi": _fold3x3(kvc_w[96:], kvdw_c_w[96:]),
        "q_lo": _fold3x3(qt_w, qdw_t_w),
        "k_lo": _fold3x3(kvt_w[:96], kvdw_t_w[:96]),
        "v_lo": _fold3x3(kvt_w[96:], kvdw_t_w[96:]),
    }
    wqk_hi = _bf16(np.concatenate([W3["q_hi"], W3["k_hi"]], axis=2))  # [9,96,192]
    wqk_lo = _bf16(np.concatenate([W3["k_lo"], W3["q_lo"]], axis=2))
    wv_hi = _bf16(W3["v_hi"])
    wv_lo = _bf16(W3["v_lo"])
    # device layout [D(ci), 9, O]
    wqk_hi = np.ascontiguousarray(wqk_hi.transpose(1, 0, 2))
    wqk_lo = np.ascontiguousarray(wqk_lo.transpose(1, 0, 2))
    wv_hi = np.ascontiguousarray(wv_hi.transpose(1, 0, 2))
    wv_lo = np.ascontiguousarray(wv_lo.transpose(1, 0, 2))
    P_c = concat_w[:, :96] @ po_c_w
    P_t = concat_w[:, 96:] @ po_t_w
    pct = _bf16(P_c.T)
    ptt = _bf16(P_t.T)
    ident = _bf16(np.eye(D, dtype=np.float32))
    tempv = np.repeat(np.asarray(temperature, np.float32).reshape(3), 32)[:, None]
    biasv = np.asarray(concat_b, np.float32)[:, None]

    # pack all bf16 weights into one flat buffer, split 8 ways
    wflat = np.concatenate([
        wqk_hi.ravel(), wqk_lo.ravel(), wv_hi.ravel(), wv_lo.ravel(),
        pct.ravel(), ptt.ravel(), ident.ravel()])
    assert wflat.size == WTOT
    wshards = [np.ascontiguousarray(wflat[c * WSH:(c + 1) * WSH].reshape(1, WSH))
               for c in range(NC)]

    lo_sh, lo_sc = _quant_in(low)
    hi_sh, hi_sc = _quant_in(high)

    in_maps = []
    for c in range(NC):
        rows = slice(c * RPC, (c + 1) * RPC)
        blob = np.empty((1, NB), np.int8)
        fl = blob[0]
        fl[0:2 * XP1] = hi_sh[c].reshape(-1).view(np.int8)
        fl[2 * XP1:4 * XP1] = lo_sh[c].reshape(-1).view(np.int8)
        fl[WOFF:WOFF + WSH * 2] = wshards[c].reshape(-1).view(np.int8)
        scr = np.stack([hi_sc[0][:, rows], hi_sc[1][:, rows],
                        lo_sc[0][:, rows], lo_sc[1][:, rows]])
        fl[SOFF:TOFF] = scr.astype(np.float32).reshape(-1).view(np.int8)
        fl[TOFF:TOFF + D * 4] = tempv.astype(np.float32).reshape(-1).view(np.int8)
        fl[TOFF + D * 4:NB] = biasv.astype(np.float32).reshape(-1).view(np.int8)
        in_maps.append({"xin": blob})
    return in_maps


def run(trace=False, in_maps=None, **inputs):
    import time as _time
    from concourse.bass_utils import run_bass_kernel_spmd
    nc = _get_nc()
    if in_maps is None:
        in_maps = _prep_inputs(**inputs)
    t0 = _time.time()
    res = run_bass_kernel_spmd(nc, in_maps, list(range(NC)), trace=trace)
    res.dispatch_wall_s = _time.time() - t0
    res.in_maps = in_maps
    out = np.empty((B, D, H, W), np.float32)
    for c in range(NC):
        raw = res.results[c]["oq8"]  # [B, D, OCOL] int8
        oscl = raw[:, :, RPC * W:].copy().view(np.float32)  # [B, D, 1]
        oi = raw[:, :, :RPC * W].astype(np.float32) * oscl
        out[:, :, c * RPC:(c + 1) * RPC, :] = oi.reshape(B, D, RPC, W)
    return out, res


def kernel(**inputs):
    out, _ = run(trace=False, **inputs)
    return out
